# revision 59
# baseline (speedup 1.0000x reference)
"""Trainium2 Bass kernel for nn_DenTargetTransformerConv (GNN message passing).

Strategy (graph/data parallel, dst-owner sharding across 8 NeuronCores):
  - Nodes are partitioned by dst-id range; each core owns N/8 nodes and all
    edges whose dst falls in its range (the "halo exchange" of src features
    is materialized host-side as per-core fp16 edge tables).
  - Per core, own nodes are sorted by in-degree and packed into groups of
    128 (SBUF partition dim). Every node in group g gets K[g] edge slots
    (K[g] = max degree at that position across all cores, so the 8 cores
    share one compiled program). Per-edge q rows are stored slot-major with
    d-major elements; v rows are d-major across the whole run. Every fetch
    is a contiguous 2D dma_start; every hot DVE op is a <=2-free-dim fp16
    access with a packed output in a different buffer than its inputs
    (in-place/bank-conflicting ops measured 2-4x slower).
  - Scores: one 2-dim mult per group + 4 out-of-place contiguous-packing
    half-folds over d. Softmax: exp with a -ln(16) bias (padding slots
    contribute exactly 1/16 to the denominator, corrected by an exact host
    pad count -- no mask). exp writes a 17th stripe of the w tile, so one
    f32 tensor_reduce yields the aggregation AND the denominator (k is
    tree-folded out-of-place while halves stay large).
  - The instruction stream is software-pipelined: run r's score phase is
    emitted before run r-1's aggregation phase, and the node phase
    (gate/LayerNorm/PReLU) is chopped into small stages drip-fed between
    runs, so in-order engines do not idle on cross-engine latency.
    Sigmoid is 1/(1+exp(-x)) and rsqrt is exp(-0.5*ln(x)), keeping every
    activation in one ACT table.
"""

import numpy as np

import concourse.bacc as bacc
import concourse.bass as bass
import concourse.tile as tile
from concourse import mybir
from concourse.bass_utils import run_bass_kernel_spmd

F32 = mybir.dt.float32
F16 = mybir.dt.float16
AX = mybir.AxisListType
ALU = mybir.AluOpType
ACTF = mybir.ActivationFunctionType

P = 128
NCORES = 8
HD = 64          # H * D
H, D = 4, 16
IN_F = 64
ST = D + 1       # stripes in the w tile: 16 d-stripes + 1 ex stripe

RUNC = 104       # max slot-columns per merged compute run
NCHUNK = 5       # node-phase chunks, each split into pipelined stage thunks
LN16 = float(np.log(16.0))


def _ap(base, offset_elems, dims):
    """AP with the partition dim of `base` and explicit free dims."""
    return bass.AP(tensor=base.tensor, offset=base.offset + offset_elems,
                   ap=[base.ap[0]] + [list(d) for d in dims])


# ----------------------------------------------------------------- host prep

def _plan(q_src, v_src, feat, src, dst, ncores):
    n = feat.shape[0]
    npc = n // ncores
    ngrp = (npc + P - 1) // P
    grid = ngrp * P
    ndum = grid - npc

    q16 = np.asarray(q_src, np.float32).reshape(n, HD).astype(np.float16)
    v16 = np.asarray(v_src, np.float32).reshape(n, HD).astype(np.float16)

    src = np.asarray(src).astype(np.int64)
    dst = np.asarray(dst).astype(np.int64)
    order = np.argsort(dst, kind="stable")
    dst_s, src_s = dst[order], src[order]
    bounds = np.searchsorted(dst_s, np.arange(ncores + 1) * npc)

    cores = []
    gmax = np.zeros((ncores, ngrp), np.int64)
    for c in range(ncores):
        lo, hi = bounds[c], bounds[c + 1]
        dstL = dst_s[lo:hi] - c * npc          # ascending
        srcL = src_s[lo:hi]
        deg = np.bincount(dstL, minlength=npc)
        starts = np.concatenate([[0], np.cumsum(deg)])
        rank = np.arange(len(dstL)) - starts[dstL]
        perm = np.argsort(deg, kind="stable")  # ascending degree
        pos_of = np.empty(npc, np.int64)
        pos_of[perm] = ndum + np.arange(npc)
        gd = np.zeros(grid, np.int64)
        gd[ndum:] = deg[perm]
        gmax[c] = gd.reshape(ngrp, P).max(1)
        cores.append(dict(dstL=dstL, srcL=srcL, rank=rank, perm=perm,
                          pos_of=pos_of, gd=gd))

    K = np.maximum(gmax.max(0), 1)             # shared per-group slot count
    # Merge consecutive groups (K is ascending) into runs of <= RUNC
    # slot-columns; every group in a run is padded to the run's max K.
    runs = []                                  # (g0, g1, Kr, colstart)
    g = 0
    cb = 0
    while g < ngrp:
        ge = g + 1
        while ge < ngrp and (ge - g + 1) * int(K[ge]) <= RUNC:
            ge += 1
        kr = int(K[ge - 1])
        runs.append((g, ge, kr, cb))
        cb += (ge - g) * kr
        g = ge
    totc = cb
    mrc = max((g1 - g0) * kr for (g0, g1, kr, c0) in runs)

    run_of_g = np.empty(ngrp, np.int64)
    kr_of_g = np.empty(ngrp, np.int64)
    g0_of_run = np.empty(len(runs), np.int64)
    cb_of_run = np.empty(len(runs), np.int64)
    rk_of_run = np.empty(len(runs), np.int64)
    for ri, (g0, g1, kr, c0) in enumerate(runs):
        run_of_g[g0:g1] = ri
        kr_of_g[g0:g1] = kr
        g0_of_run[ri] = g0
        cb_of_run[ri] = c0
        rk_of_run[ri] = (g1 - g0) * kr

    # Per-core fp16 edge table: per run-slab [q block: RK*64, slot-major
    # with (d,h) elements | v block: RK*64, d-major], partition-major rows.
    per_core = []
    j64 = np.arange(64)
    off_d = (j64 % D).astype(np.int64)         # column j of q16/v16 is (h, d)
    off_h = (j64 // D).astype(np.int64)
    qoff = off_d * H + off_h                   # in-slot d-major
    for c in range(ncores):
        cd = cores[c]
        pos_e = cd["pos_of"][cd["dstL"]]
        g_e = pos_e // P
        p_e = pos_e % P
        r_e = run_of_g[g_e]
        slab_e = cb_of_run[r_e] * 2 * HD
        crel_e = (g_e - g0_of_run[r_e]) * kr_of_g[g_e] + cd["rank"]
        rk_e = rk_of_run[r_e]
        base_e = p_e * (totc * 2 * HD) + slab_e
        tabf = np.zeros(P * totc * 2 * HD, np.float16)
        tabf[(base_e + crel_e * HD)[:, None] + qoff[None, :]] = \
            q16[cd["srcL"]]
        vmaj = off_d[None, :] * (rk_e[:, None] * H) + off_h[None, :]
        tabf[(base_e + rk_e * HD + crel_e * H)[:, None] + vmaj] = \
            v16[cd["srcL"]]
        tab = tabf.reshape(P, totc * 2 * HD)
        # denominator correction: padded slots contribute exp(-ln16)=1/16
        sub = ((kr_of_g[None, :] - cd["gd"].reshape(ngrp, P).T)
               .astype(np.float64) / 16.0 - 1e-9).astype(np.float32)
        per_core.append(dict(tab=tab, sub=sub))

    # featT with ones row, per core, grid-permuted: [IN_F+1, grid] fp16
    featTs = []
    feat = np.asarray(feat, np.float32)
    for c in range(ncores):
        ft = np.zeros((IN_F + 1, grid), np.float16)
        ft[IN_F, :] = 1.0
        perm = cores[c]["perm"]
        ft[:IN_F, ndum:] = feat[c * npc + perm].T.astype(np.float16)
        featTs.append(ft)

    return dict(n=n, npc=npc, ngrp=ngrp, grid=grid, ndum=ndum, K=K,
                totc=totc, runs=runs, mrc=mrc,
                cores=cores, per_core=per_core, featTs=featTs)


# ------------------------------------------------------------- device build

def _build_nc(plan, ncores):
    ngrp, totc, runs = plan["ngrp"], plan["totc"], plan["runs"]
    grid = plan["grid"]
    mrc = plan["mrc"]
    nruns = len(runs)

    nc = bacc.Bacc("TRN2", target_bir_lowering=False, debug=False,
                   num_devices=ncores)

    tab_d = nc.dram_tensor("tab", [P, totc * 2 * HD], F16,
                           kind="ExternalInput").ap()
    featT_d = nc.dram_tensor("featT", [IN_F + 1, grid], F16,
                             kind="ExternalInput").ap()
    wks_d = nc.dram_tensor("wks", [IN_F + 1, 2 * HD], F16,
                           kind="ExternalInput").ap()
    # fp16 params: [wg1' (64) | wg2' (64) | gamma (64) | beta (64)]
    par16_d = nc.dram_tensor("par16", [1, 256], F16, kind="ExternalInput").ap()
    # f32 params: [prelu_a, -bgate]
    par_d = nc.dram_tensor("par", [1, 2], F32, kind="ExternalInput").ap()
    sub_d = nc.dram_tensor("sub", [P, ngrp], F32, kind="ExternalInput").ap()
    out_d = nc.dram_tensor("out", [P, ngrp * HD], F32,
                           kind="ExternalOutput").ap()

    with tile.TileContext(nc) as tc:
        with (
            tc.tile_pool(name="singles", bufs=1) as singles,
            tc.tile_pool(name="psum", bufs=4, space="PSUM") as psum,
            tc.tile_pool(name="qtp", bufs=2) as qtp,
            tc.tile_pool(name="vtp", bufs=2) as vtp,
            tc.tile_pool(name="wp", bufs=2) as wp,
            tc.tile_pool(name="w2p", bufs=2) as w2p,
        ):
            # ---- static loads (wks/featT first: the PE preamble needs them)
            wks = singles.tile([IN_F + 1, 2 * HD], F16)
            nc.sync.dma_start(out=wks[:], in_=wks_d[:])
            # prefetch run 0's q table ahead of the big featT load so the
            # first score mults can start as early as possible
            tiles = {}
            g0e, g1e, Ke, c0e = runs[0]
            RKe = (g1e - g0e) * Ke
            qt0 = qtp.tile([P, mrc * HD], F16, tag="qt")
            vt0 = vtp.tile([P, mrc * HD], F16, tag="vt")
            w_t0 = wp.tile([P, mrc * ST * H], F16, tag="w")
            w20 = w2p.tile([P, mrc * 60], F16, tag="w2")
            tiles[0] = (qt0, vt0, w_t0, w20)
            nc.sync.dma_start(
                out=qt0[:, :RKe * HD],
                in_=tab_d[:, c0e * 2 * HD:c0e * 2 * HD + RKe * HD])
            # featT in two pieces: the first slice unblocks the early PE
            # matmuls (and so the first score mults) a few us sooner
            featT = singles.tile([IN_F + 1, grid], F16)
            fsp = min(12 * P, grid)
            nc.sync.dma_start(out=featT[:, :fsp], in_=featT_d[:, :fsp])
            nc.sync.dma_start(out=featT[:, fsp:], in_=featT_d[:, fsp:])
            sub_sb = singles.tile([P, ngrp], F32)
            nc.sync.dma_start(out=sub_sb[:], in_=sub_d[:])
            parb16 = singles.tile([P, 256], F16)
            nc.gpsimd.dma_start(
                out=parb16[:],
                in_=bass.AP(tensor=par16_d.tensor, offset=par16_d.offset,
                            ap=[[0, P], [1, 256]]))
            parb = singles.tile([P, 2], F32)
            nc.gpsimd.dma_start(
                out=parb[:],
                in_=bass.AP(tensor=par_d.tensor, offset=par_d.offset,
                            ap=[[0, P], [1, 2]]))
            wg1 = parb16[:, 0:64]
            wg2 = parb16[:, 64:128]
            gamma = parb16[:, 128:192]
            beta = parb16[:, 192:256]
            pa = parb[:, 0:1]
            nbg = parb[:, 1:2]

            eps_t = singles.tile([P, 1], F32)
            nc.vector.memset(eps_t[:], 1e-5)
            nln16_t = singles.tile([P, 1], F32)
            nc.vector.memset(nln16_t[:], -LN16)
            one_t = singles.tile([P, 1], F32)
            nc.vector.memset(one_t[:], 1.0)

            # ---- per-node linears on PE: ks = [k | skip] per group, fp16
            # (k columns are stored d-major: col d*4+h)
            ks = singles.tile([P, ngrp * 2 * HD], F16)
            for g in range(ngrp):
                pk = psum.tile([P, 2 * HD], F32, tag="pk")
                nc.tensor.matmul(out=pk[:], lhsT=featT[:, g * P:(g + 1) * P],
                                 rhs=wks[:], start=True, stop=True)
                nc.scalar.activation(out=ks[:, g * 128:(g + 1) * 128],
                                     in_=pk[:], func=ACTF.Copy)

            agg_sb = singles.tile([P, ngrp * ST * H], F32)
            rst = singles.tile([P, ngrp * HD], F16)
            zt = singles.tile([P, ngrp * HD], F16)
            zt2 = singles.tile([P, ngrp * HD], F16)
            outb = singles.tile([P, ngrp * HD], F32)
            gl = singles.tile([P, ngrp], F32)
            mu = singles.tile([P, ngrp], F32)
            var = singles.tile([P, ngrp], F32)

            # ---------------- node phase: thunk lists, one DVE op per thunk
            # (plus trailing same-dependency-chain ops on other engines)
            def stage_thunks(ga, gb):
                NG = gb - ga
                skipv = _ap(ks[:], ga * 128 + HD, [[128, NG], [1, HD]])
                r2 = _ap(rst[:], ga * HD, [[HD, NG], [1, HD]])
                r1 = _ap(rst[:], ga * HD, [[1, NG * HD]])
                z1 = _ap(zt[:], ga * HD, [[1, NG * HD]])
                z21 = _ap(zt2[:], ga * HD, [[1, NG * HD]])
                z2 = _ap(zt[:], ga * HD, [[HD, NG], [1, HD]])
                z22 = _ap(zt2[:], ga * HD, [[HD, NG], [1, HD]])

                def a1():
                    # gate logits on pool; sigmoid = exp(-ln(exp(-x-bg)+1))
                    nc.gpsimd.tensor_tensor(
                        out=z2, in0=skipv,
                        in1=_ap(wg1, 0, [[0, NG], [1, HD]]), op=ALU.mult)
                    nc.gpsimd.tensor_tensor(
                        out=z22, in0=r2,
                        in1=_ap(wg2, 0, [[0, NG], [1, HD]]), op=ALU.mult)
                    nc.gpsimd.tensor_tensor(out=z1, in0=z1, in1=z21,
                                            op=ALU.add)
                    nc.vector.tensor_reduce(out=gl[:, ga:gb], in_=z2,
                                            axis=AX.X, op=ALU.add)
                    nc.scalar.activation(out=gl[:, ga:gb], in_=gl[:, ga:gb],
                                         func=ACTF.Exp, scale=-1.0, bias=nbg)
                    nc.gpsimd.tensor_scalar(out=gl[:, ga:gb],
                                            in0=gl[:, ga:gb], scalar1=1.0,
                                            scalar2=None, op0=ALU.add)

                def a2():
                    nc.vector.reciprocal(out=gl[:, ga:gb], in_=gl[:, ga:gb])

                def b1():
                    nc.vector.tensor_tensor(out=z22, in0=skipv, in1=r2,
                                            op=ALU.subtract)

                def b2():
                    nc.vector.tensor_tensor(
                        out=z22, in0=z22,
                        in1=_ap(gl[:], ga, [[1, NG], [0, HD]]),
                        op=ALU.mult)

                def b3():
                    nc.vector.tensor_tensor(out=r1, in0=r1, in1=z21,
                                            op=ALU.add)

                def b4():
                    nc.vector.tensor_reduce(out=mu[:, ga:gb], in_=r2,
                                            axis=AX.X, op=ALU.add)
                    nc.scalar.activation(out=mu[:, ga:gb], in_=mu[:, ga:gb],
                                         func=ACTF.Copy, scale=1.0 / HD)

                def b5():
                    nc.vector.tensor_tensor(
                        out=r2, in0=r2,
                        in1=_ap(mu[:], ga, [[1, NG], [0, HD]]),
                        op=ALU.subtract)
                    nc.scalar.activation(out=z21, in_=r1, func=ACTF.Square)

                def c1():
                    nc.vector.tensor_reduce(out=var[:, ga:gb], in_=z22,
                                            axis=AX.X, op=ALU.add)
                    # rsqrt(var/64 + eps) = exp(-0.5 * ln(var/64 + eps))
                    nc.scalar.activation(out=var[:, ga:gb], in_=var[:, ga:gb],
                                         func=ACTF.Ln, scale=1.0 / HD,
                                         bias=eps_t[:])
                    nc.scalar.activation(out=var[:, ga:gb], in_=var[:, ga:gb],
                                         func=ACTF.Exp, scale=-0.5)

                def c2():
                    nc.vector.tensor_tensor(
                        out=r2, in0=r2,
                        in1=_ap(var[:], ga, [[1, NG], [0, HD]]),
                        op=ALU.mult)
                    nc.gpsimd.tensor_tensor(
                        out=r2, in0=r2,
                        in1=_ap(gamma, 0, [[0, NG], [1, HD]]), op=ALU.mult)
                    nc.gpsimd.tensor_tensor(
                        out=r2, in0=r2,
                        in1=_ap(beta, 0, [[0, NG], [1, HD]]), op=ALU.add)
                    nc.scalar.activation(
                        out=_ap(outb[:], ga * HD, [[1, NG * HD]]),
                        in_=r1, func=ACTF.Prelu, alpha=pa)
                    nc.sync.dma_start(out=out_d[:, ga * HD:gb * HD],
                                      in_=outb[:, ga * HD:gb * HD])

                return [a1, a2, b1, b2, b3, b4, b5, c1, c2]

            # ---------------- edge phase thunk builders
            def score_thunks(ri):
                g0, g1, K, c0 = runs[ri]
                R = g1 - g0
                RK = R * K
                SP = RK * H
                qt, vt, w_t, w2 = tiles[ri]
                qvb, wb, w2b = qt[:, 0:1], w_t[:, 0:1], w2[:, 0:1]
                out = []
                for gg in range(R):
                    def mult(gg=gg):
                        nc.vector.tensor_tensor(
                            out=_ap(wb, gg * K * HD, [[HD, K], [1, HD]]),
                            in0=_ap(qvb, gg * K * HD, [[HD, K], [1, HD]]),
                            in1=_ap(ks[:], (g0 + gg) * 128,
                                    [[0, K], [1, HD]]),
                            op=ALU.mult)
                    out.append(mult)
                # fold d out-of-place: slot blocks 64 -> 32 -> 16 -> 8 -> 4
                lay = [(wb, 0, HD), (w2b, 0, 32), (w2b, RK * 32, 16),
                       (w2b, RK * 48, 8), (w2b, RK * 56, 4)]
                for li in range(4):
                    def fold(li=li):
                        sb_i, off_i, blk = lay[li]
                        sb_o, off_o, _ = lay[li + 1]
                        hb = blk // 2
                        nc.vector.tensor_tensor(
                            out=_ap(sb_o, off_o, [[1, RK * hb]]),
                            in0=_ap(sb_i, off_i, [[blk, RK], [1, hb]]),
                            in1=_ap(sb_i, off_i + hb, [[blk, RK], [1, hb]]),
                            op=ALU.add)
                        if li == 3:
                            # ex = exp(score/4 - ln16) -> stripe 16 of w
                            nc.scalar.activation(
                                out=_ap(wb, D * SP, [[1, SP]]),
                                in_=_ap(w2b, RK * 56, [[1, SP]]),
                                func=ACTF.Exp, scale=0.25, bias=nln16_t[:])
                    out.append(fold)
                return out

            def agg_thunks(ri):
                g0, g1, K, c0 = runs[ri]
                R = g1 - g0
                RK = R * K
                SP = RK * H
                qt, vt, w_t, w2 = tiles[ri]
                vtb, wb, w2b = vt[:, 0:1], w_t[:, 0:1], w2[:, 0:1]
                out = []

                def wmult():
                    nc.vector.tensor_tensor(
                        out=_ap(wb, 0, [[1, RK * HD]]),
                        in0=_ap(vtb, 0, [[1, RK * HD]]),
                        in1=_ap(wb, D * SP, [[0, D], [1, SP]]),
                        op=ALU.mult)
                out.append(wmult)
                # fold k out-of-place (ping-pong w <-> w2) while m > 6
                m = K
                cur, coff = wb, 0
                other, ooff = w2b, 0
                while m > 6:
                    hh = (m + 1) // 2
                    nf = m - hh

                    def fk(m=m, hh=hh, nf=nf, cur=cur, coff=coff,
                           other=other, ooff=ooff):
                        nc.vector.tensor_tensor(
                            out=_ap(other, ooff,
                                    [[hh * H, ST * R], [1, H * nf]]),
                            in0=_ap(cur, coff, [[m * H, ST * R], [1, H * nf]]),
                            in1=_ap(cur, coff + hh * H,
                                    [[m * H, ST * R], [1, H * nf]]),
                            op=ALU.add)
                        if nf < hh:    # odd m: carry the middle slot over
                            nc.gpsimd.tensor_copy(
                                out=_ap(other, ooff + nf * H,
                                        [[hh * H, ST * R], [1, H]]),
                                in_=_ap(cur, coff + nf * H,
                                        [[m * H, ST * R], [1, H]]))
                    out.append(fk)
                    m = hh
                    cur, coff, other, ooff = other, ooff, cur, coff

                def tr(m=m, cur=cur, coff=coff):
                    nc.vector.tensor_reduce(
                        out=_ap(agg_sb[:], g0 * ST * H, [[1, ST * R * H]]),
                        in_=_ap(cur, coff, [[m * H, ST * R], [1, H], [H, m]]),
                        axis=AX.X, op=ALU.add)
                    dofs = g0 * ST * H + D * R * H
                    nc.gpsimd.tensor_tensor(
                        out=_ap(agg_sb[:], dofs, [[H, R], [1, H]]),
                        in0=_ap(agg_sb[:], dofs, [[H, R], [1, H]]),
                        in1=_ap(sub_sb[:], g0, [[1, R], [0, H]]),
                        op=ALU.subtract)
                out.append(tr)

                def fin():
                    dofs = g0 * ST * H + D * R * H
                    nc.vector.reciprocal(
                        out=_ap(agg_sb[:], dofs, [[1, R * H]]),
                        in_=_ap(agg_sb[:], dofs, [[1, R * H]]))
                    # rst[g-major] = agg[d-major] * dinv (transposing op)
                    nc.gpsimd.tensor_tensor(
                        out=_ap(rst[:], g0 * HD, [[HD, R], [D, H], [1, D]]),
                        in0=_ap(agg_sb[:], g0 * ST * H,
                                [[H, R], [1, H], [R * H, D]]),
                        in1=_ap(agg_sb[:], dofs,
                                [[H, R], [1, H], [0, D]]),
                        op=ALU.mult)
                out.append(fin)
                return out

            # ---------------- pipelined, instruction-interleaved emission
            bnds = [int(np.ceil(ngrp * (i + 1) / NCHUNK))
                    for i in range(NCHUNK)]

            def issue_qdma(ri):
                g0, g1, K, c0 = runs[ri]
                RK = (g1 - g0) * K
                slab = c0 * 2 * HD
                qt = qtp.tile([P, mrc * HD], F16, tag="qt")
                vt = vtp.tile([P, mrc * HD], F16, tag="vt")
                w_t = wp.tile([P, mrc * ST * H], F16, tag="w")
                w2 = w2p.tile([P, mrc * 60], F16, tag="w2")
                tiles[ri] = (qt, vt, w_t, w2)
                nc.sync.dma_start(
                    out=qt[:, :RK * HD],
                    in_=tab_d[:, slab:slab + RK * HD])

            def issue_vdma(ri):
                g0, g1, K, c0 = runs[ri]
                RK = (g1 - g0) * K
                slab = c0 * 2 * HD + RK * HD
                vt = tiles[ri][1]
                nc.sync.dma_start(
                    out=vt[:, :RK * HD],
                    in_=tab_d[:, slab:slab + RK * HD])

            stageq = []                 # pending node-phase thunks
            done = 0
            bi = 0
            for ri in range(nruns):
                if ri + 1 < nruns:
                    issue_qdma(ri + 1)
                issue_vdma(ri)
                sc = score_thunks(ri)
                ag = agg_thunks(ri - 1) if ri > 0 else []
                # alternate score(ri) and [agg(ri-1) + node] ops so adjacent
                # DVE instructions come from independent dependency chains
                fill = ag + stageq[:max(0, len(sc) - len(ag))]
                stageq = stageq[max(0, len(sc) - len(ag)):]
                n = max(len(sc), len(fill))
                for i in range(n):
                    if i < len(sc):
                        sc[i]()
                    if i < len(fill):
                        fill[i]()
                if ri > 0:
                    g1p = runs[ri - 1][1]
                    if bi < NCHUNK and g1p >= bnds[bi]:
                        stageq += stage_thunks(done, g1p)
                        done = g1p
                        while bi < NCHUNK and bnds[bi] <= done:
                            bi += 1
            ag = agg_thunks(nruns - 1)
            if done < ngrp:
                # split the final chunk into two independent half-range
                # chains and zip them, so the tail always has a
                # non-dependent op to hide completion latency behind
                mid = (done + ngrp + 1) // 2
                if mid > done and mid < ngrp:
                    s1 = stage_thunks(done, mid)
                    s2 = stage_thunks(mid, ngrp)
                    for t1, t2 in zip(s1, s2):
                        stageq += [t1, t2]
                else:
                    stageq += stage_thunks(done, ngrp)
            # tail: alternate the two independent chains
            n = max(len(ag), len(stageq))
            for i in range(n):
                if i < len(ag):
                    ag[i]()
                if i < len(stageq):
                    stageq[i]()

    nc.compile()
    return nc


# ------------------------------------------------------------------- driver

_CACHE = {}


def _get_nc(plan, ncores):
    key = (tuple(plan["K"].tolist()), plan["grid"], plan["totc"], ncores)
    if key not in _CACHE:
        _CACHE[key] = _build_nc(plan, ncores)
    return _CACHE[key]


def _make_inmaps(plan, params, ncores):
    (Wk, bk, Wskip, bskip, Wgate, bgate, ln_gamma, ln_beta, prelu_a) = params
    # k columns permuted to d-major (col d*4+h <- col h*16+d)
    j = np.arange(HD)
    kperm = np.empty(HD, np.int64)
    kperm[(j % D) * H + j // D] = j
    wks = np.zeros((IN_F + 1, 2 * HD), np.float16)
    wks[:IN_F, 0:HD] = np.asarray(Wk, np.float32).astype(np.float16)[:, kperm]
    wks[IN_F, 0:HD] = np.asarray(bk, np.float32).astype(np.float16)[kperm]
    wks[:IN_F, HD:] = np.asarray(Wskip, np.float32).astype(np.float16)
    wks[IN_F, HD:] = np.asarray(bskip, np.float32).astype(np.float16)
    wg = np.asarray(Wgate, np.float32).reshape(3 * HD)
    par16 = np.zeros((1, 256), np.float16)
    par16[0, 0:64] = (wg[0:64] + wg[128:192]).astype(np.float16)
    par16[0, 64:128] = (wg[64:128] - wg[128:192]).astype(np.float16)
    par16[0, 128:192] = np.asarray(ln_gamma, np.float32).astype(np.float16)
    par16[0, 192:256] = np.asarray(ln_beta, np.float32).astype(np.float16)
    par = np.zeros((1, 2), np.float32)
    par[0, 0] = np.float32(np.asarray(prelu_a).reshape(-1)[0])
    par[0, 1] = -np.float32(np.asarray(bgate).reshape(-1)[0])

    in_maps = []
    for c in range(ncores):
        pc = plan["per_core"][c]
        in_maps.append(dict(tab=pc["tab"], sub=pc["sub"],
                            featT=plan["featTs"][c],
                            wks=wks, par16=par16, par=par))
    return in_maps


def run(q_src, v_src, feat, src, dst, Wk, bk, Wskip, bskip, Wgate, bgate,
        ln_gamma, ln_beta, prelu_a, ncores=NCORES, trace=False):
    plan = _plan(q_src, v_src, feat, src, dst, ncores)
    nc = _get_nc(plan, ncores)
    in_maps = _make_inmaps(
        plan, (Wk, bk, Wskip, bskip, Wgate, bgate, ln_gamma, ln_beta, prelu_a),
        ncores)
    res = run_bass_kernel_spmd(nc, in_maps, core_ids=list(range(ncores)),
                               trace=trace)
    n, npc, ngrp = plan["n"], plan["npc"], plan["ngrp"]
    out = np.empty((n, HD), np.float32)
    for c in range(ncores):
        r = res.results[c]["out"]                          # [128, ngrp*64]
        arr = r.reshape(P, ngrp, HD).transpose(1, 0, 2).reshape(-1, HD)
        out[c * npc + plan["cores"][c]["perm"]] = \
            arr[plan["ndum"]:plan["ndum"] + npc]
    return out, res, plan, in_maps, nc


def kernel(**inputs):
    out, _, _, _, _ = run(**inputs)
    return out
